# revision 9
# baseline (speedup 1.0000x reference)
"""Trainium2 Bass kernel for nn_Attention_7919919694519.

Multi-head attention (B=2, L=2048, H=16, d=64) with two data-dependent masks:
  - V_len[b] masks HEADS h >= V_len[b]: the reference adds -1e12 to every
    score of those heads, which collapses (in fp32) to a uniform softmax, so
    the masked head's output is mean_k(v) = (mean_k V_seq) @ WV_h  (rank-1).
  - Q_len[b] zeroes output rows q >= Q_len[b].

Strategy (host-visible Q_len/V_len drive the work list):
  - Only unmasked heads with live q rows do real attention. Each unmasked
    head is a "job" needing ceil(Q_len[b]/512) q-chunks (the last chunk
    trimmed to a 128-multiple of its live rows). Jobs are packed into
    head-slots dealt across 8 NeuronCores (SPMD: same NEFF, different
    data); K/V are projected once per slot, q-chunks stream through.
    No collectives; host scatters/gathers.
  - Per chunk on device: project q (bf16 matmul), scores S^T[k,q] in PSUM,
    exp on ScalarE (PSUM->SBUF bf16), AV accumulation with a ones-column
    appended to v so softmax denominators fall out of the same matmuls,
    PE transpose, reciprocal+scale on VectorE, DMA out. Emission is
    software-pipelined across chunk-units so ScalarE (the bottleneck
    engine) never starves at unit boundaries.
  - Masked-head rank-1 content: device reduces V_seq over k (VectorE) and
    projects through WV/2048; host broadcasts rows (pure output assembly).
"""

import math
import numpy as np
import ml_dtypes

import concourse.bass as bass
import concourse.tile as tile
from concourse import bacc, mybir
from concourse.bass_utils import run_bass_kernel_spmd
from concourse.masks import make_identity
from contextlib import ExitStack

BF16 = ml_dtypes.bfloat16
N_CORES = 8
B_, L_, D_, H_ = 2, 2048, 64, 16
NQ = 512              # max q rows per chunk
KT = 16               # number of 128-row k tiles (L/128)
SPS_FD = 1536         # score-psum slot free dim (3 banks)

_cache = {}


def _chunk_plan(nq):
    """k-tile sub-chunk sizes for a q-width nq (each <= SPS_FD elems)."""
    cl = max(1, SPS_FD // nq)
    out = [cl] * (KT // cl)
    if KT % cl:
        out.append(KT % cl)
    return out


def _build(struct):
    """Build + compile the SPMD NEFF.

    struct: tuple of per-slot tuples of chunk q-widths, e.g.
    ((512, 512, 512, 128), (512, 512, 256))."""
    nc = bacc.Bacc("TRN2", target_bir_lowering=False, debug=False,
                   num_devices=N_CORES)
    dt = mybir.dt
    S = len(struct)
    units = []   # flat: (slot, first_in_slot, q_width)
    for s in range(S):
        for r, nq in enumerate(struct[s]):
            units.append((s, r == 0, nq))
    NU = len(units)

    qt_d = nc.dram_tensor("qt", [NU, 64, NQ], dt.bfloat16, kind="ExternalInput").ap()
    kt_d = nc.dram_tensor("kt", [S, 64, L_], dt.bfloat16, kind="ExternalInput").ap()
    vt_d = nc.dram_tensor("vt", [S, 64, L_], dt.bfloat16, kind="ExternalInput").ap()
    w_d = nc.dram_tensor("w", [S, 64, 192], dt.bfloat16, kind="ExternalInput").ap()
    vtb_d = nc.dram_tensor("vtb", [B_, 64, L_], dt.float32, kind="ExternalInput").ap()
    wvm_d = nc.dram_tensor("wvm", [64, H_ * 64], dt.float32, kind="ExternalInput").ap()
    out_d = nc.dram_tensor("out", [NU, NQ, 64], dt.float32, kind="ExternalOutput").ap()
    mo_d = nc.dram_tensor("meanout", [128, 8, B_], dt.float32, kind="ExternalOutput").ap()

    with tile.TileContext(nc) as tc, ExitStack() as ctx:
        inp = ctx.enter_context(tc.tile_pool(name="inp", bufs=2))
        proj = ctx.enter_context(tc.tile_pool(name="proj", bufs=2))
        expp = ctx.enter_context(tc.tile_pool(name="expp", bufs=3))
        ob = ctx.enter_context(tc.tile_pool(name="ob", bufs=3))
        single = ctx.enter_context(tc.tile_pool(name="single", bufs=1))
        ps_s = ctx.enter_context(tc.tile_pool(name="ps_s", bufs=2, space="PSUM"))
        ps_a = ctx.enter_context(tc.tile_pool(name="ps_a", bufs=2, space="PSUM"))

        ident = single.tile([128, 128], dt.float32)
        make_identity(nc, ident[:])

        st = [dict() for _ in range(NU)]
        slot_tiles = {}

        def slot_k_prologue(u):
            s, first, _ = units[u]
            if not first:
                return
            kt_sb = inp.tile([64, L_], dt.bfloat16, tag="kt")
            nc.sync.dma_start(kt_sb[:], kt_d[s])
            vt_sb = inp.tile([64, L_], dt.bfloat16, tag="vt")
            nc.sync.dma_start(vt_sb[:], vt_d[s])
            w_sb = inp.tile([64, 192], dt.bfloat16, tag="w")
            nc.sync.dma_start(w_sb[:], w_d[s])

            # k projection: kTh[64, L] = (K_seq WK_h)^T
            kTh = proj.tile([64, L_], dt.bfloat16, tag="kTh")
            for half in range(2):
                kps = ps_s.tile([64, 1024], dt.float32, tag="ps")
                for j in range(2):
                    nc.tensor.matmul(kps[:, j * 512:(j + 1) * 512],
                                     w_sb[:, 64:128],
                                     kt_sb[:, half * 1024 + j * 512:
                                           half * 1024 + (j + 1) * 512],
                                     start=True, stop=True)
                nc.vector.tensor_copy(kTh[:, half * 1024:(half + 1) * 1024],
                                      kps[:])
            slot_tiles[s] = [w_sb, kTh, None, vt_sb]

        def slot_v_prologue(u):
            s, first, _ = units[u]
            if not first:
                return
            w_sb, kTh, _, vt_sb = slot_tiles[s]
            # v projection into [k=128, 16, 65] layout (col 64 = ones)
            v_sb = proj.tile([128, KT, 65], dt.bfloat16, tag="v_sb")
            for half in range(2):
                vps = ps_s.tile([128, 8 * 64], dt.float32, tag="ps")
                for j in range(8):
                    t = half * 8 + j
                    nc.tensor.matmul(vps[:, j * 64:(j + 1) * 64],
                                     vt_sb[:, t * 128:(t + 1) * 128],
                                     w_sb[:, 128:192], start=True, stop=True)
                nc.vector.tensor_copy(
                    v_sb[:, half * 8:(half + 1) * 8, 0:64],
                    vps[:].rearrange("p (t d) -> p t d", t=8))
            nc.vector.memset(v_sb[:, :, 64], 1.0)
            slot_tiles[s][2] = v_sb

        def unit_prologue(u):
            s, _, nq = units[u]
            w_sb, kTh, v_sb, _ = slot_tiles[s]
            d = st[u]
            d["kTh"], d["s"] = kTh, s
            d["chunks"] = _chunk_plan(nq)
            d["offs"] = [sum(d["chunks"][:i]) for i in range(len(d["chunks"]) + 1)]
            d["nq"] = nq
            qt_sb = inp.tile([64, nq], dt.bfloat16, tag="qt", name=f"qt{u}")
            nc.sync.dma_start(qt_sb[:], qt_d[u][:, 0:nq])
            qps = ps_a.tile([64, nq], dt.float32, tag="pa", name=f"qps{u}")
            nc.tensor.matmul(qps[:], w_sb[:, 0:64], qt_sb[:],
                             start=True, stop=True)
            qTh = proj.tile([64, nq], dt.bfloat16, tag="qTh", name=f"qTh{u}")
            nc.vector.tensor_copy(qTh[:], qps[:])
            d["qTh"] = qTh
            d["sps"] = [None] * len(d["chunks"])
            d["ex"] = [None] * len(d["chunks"])

        def s_chunk(u, c):
            d = st[u]
            cl, nq = d["chunks"][c], d["nq"]
            sps = ps_s.tile([128, cl * nq], dt.float32, tag="ps", name=f"sps{u}_{c}")
            for j in range(cl):
                t = d["offs"][c] + j
                nc.tensor.matmul(sps[:, j * nq:(j + 1) * nq],
                                 d["kTh"][:, t * 128:(t + 1) * 128],
                                 d["qTh"][:], start=True, stop=True)
            d["sps"][c] = sps

        def e_chunk(u, c):
            d = st[u]
            cl, nq = d["chunks"][c], d["nq"]
            ex = expp.tile([128, cl * nq], dt.bfloat16, tag="ex", name=f"ex{u}_{c}")
            nc.scalar.activation(ex[:], d["sps"][c][:],
                                 mybir.ActivationFunctionType.Exp)
            d["ex"][c] = ex

        def av_chunk(u, c):
            d = st[u]
            nq = d["nq"]
            v_sb = slot_tiles[d["s"]][2]
            if c == 0:
                d["av"] = ps_a.tile([65, nq], dt.float32, tag="pa", name=f"av{u}")
            for j in range(d["chunks"][c]):
                t = d["offs"][c] + j
                nc.tensor.matmul(d["av"][:], v_sb[:, t, :],
                                 d["ex"][c][:, j * nq:(j + 1) * nq],
                                 start=(t == 0), stop=(t == KT - 1))

        def epilogue(u):
            d = st[u]
            nq = d["nq"]
            o_sb = ob.tile([65, nq], dt.float32, tag="o_sb", name=f"osb{u}")
            nc.vector.tensor_copy(o_sb[:], d["av"][:])
            for j in range(nq // 128):
                tp = ps_a.tile([128, 65], dt.float32, tag="pa", name=f"tp{u}_{j}")
                nc.tensor.transpose(tp[:], o_sb[:, j * 128:(j + 1) * 128],
                                    ident[0:65, 0:65])
                rs = ob.tile([128, 1], dt.float32, tag="rs", name=f"rs{u}_{j}")
                nc.vector.reciprocal(rs[:], tp[:, 64:65])
                ot = ob.tile([128, 64], dt.float32, tag="ot", name=f"ot{u}_{j}")
                nc.vector.tensor_scalar_mul(ot[:], tp[:, 0:64], rs[:])
                nc.sync.dma_start(out_d[u, j * 128:(j + 1) * 128, :], ot[:])
            st[u] = None

        def mean_block():
            # masked-head rank-1 content: (sum_k V_seq) @ (WV/2048)
            wvm_sb = single.tile([64, H_ * 64], dt.float32)
            nc.sync.dma_start(wvm_sb[:], wvm_d[:])
            mvt = single.tile([64, B_], dt.float32)
            for b in range(B_):
                vtb_sb = inp.tile([64, L_], dt.float32, tag="vtb")
                nc.sync.dma_start(vtb_sb[:], vtb_d[b])
                nc.vector.reduce_sum(mvt[:, b:b + 1], vtb_sb[:],
                                     axis=mybir.AxisListType.X)
            mo_sb = single.tile([128, 8, B_], dt.float32)
            for c in range(8):
                mps = ps_a.tile([128, B_], dt.float32, tag="pa", name=f"mps{c}")
                nc.tensor.matmul(mps[:], wvm_sb[:, c * 128:(c + 1) * 128], mvt[:],
                                 start=True, stop=True)
                nc.vector.tensor_copy(mo_sb[:, c, :], mps[:])
            nc.sync.dma_start(mo_d[:], mo_sb[:])

        # software pipeline across chunk-units: the next unit's prologue and
        # first score chunk are emitted before this unit's AV tail/epilogue so
        # ScalarE never starves at unit boundaries.
        slot_k_prologue(0)
        unit_prologue(0)
        slot_v_prologue(0)
        s_chunk(0, 0)
        e_chunk(0, 0)
        for u in range(NU):
            nch = len(st[u]["chunks"])
            for c in range(nch):
                if c + 1 < nch:
                    s_chunk(u, c + 1)
                    e_chunk(u, c + 1)
                    if c == nch - 2 and u + 1 < NU:
                        slot_k_prologue(u + 1)
                        unit_prologue(u + 1)
                        slot_v_prologue(u + 1)
                elif u + 1 < NU:
                    s_chunk(u + 1, 0)
                    e_chunk(u + 1, 0)
                av_chunk(u, c)
            epilogue(u)
            if u == 0:
                mean_block()

    nc.compile()
    return nc


def _round128(x):
    return max(128, (x + 127) // 128 * 128)


def _plan(q_len, v_len, B, L, H):
    """Pack unmasked-head jobs into head-slots.

    Returns (struct, assign): struct[s] = tuple of chunk q-widths;
    assign[(core, s)] = (b, h) or None."""
    jobs = []
    for b in range(B):
        nq = min(max(q_len[b], 0), L)
        nh = min(max(v_len[b], 0), H)
        if nq <= 0:
            continue
        r = (nq + NQ - 1) // NQ
        for h in range(nh):
            jobs.append((r, nq, b, h))
    jobs.sort(key=lambda x: (-x[0], -x[1]))
    n_slots = max(1, (len(jobs) + N_CORES - 1) // N_CORES)
    struct = []
    assign = {}
    for s in range(n_slots):
        col = jobs[s * N_CORES:(s + 1) * N_CORES]
        rmax = col[0][0] if col else 1
        widths = []
        for r in range(rmax):
            live = max((min(NQ, nq - r * NQ) for (jr, nq, _, _) in col
                        if r < jr), default=128)
            widths.append(_round128(live))
        struct.append(tuple(widths))
        for c in range(N_CORES):
            assign[(c, s)] = (col[c][2], col[c][3]) if c < len(col) else None
    return tuple(struct), assign


def kernel(Q_seq, K_seq, V_seq, WQ, WK, WV, Q_len, V_len):
    Q_seq = np.asarray(Q_seq, dtype=np.float32)
    K_seq = np.asarray(K_seq, dtype=np.float32)
    V_seq = np.asarray(V_seq, dtype=np.float32)
    WQ = np.asarray(WQ, dtype=np.float32)
    WK = np.asarray(WK, dtype=np.float32)
    WV = np.asarray(WV, dtype=np.float32)
    q_len = [int(x) for x in np.asarray(Q_len).reshape(-1)]
    v_len = [int(x) for x in np.asarray(V_len).reshape(-1)]
    B, L, d = Q_seq.shape
    H = WQ.shape[1] // d
    scale = 1.0 / math.sqrt(d)

    struct, assign = _plan(q_len, v_len, B, L, H)
    S = len(struct)
    offs = [0]
    for s in range(S):
        offs.append(offs[-1] + len(struct[s]))
    NU = offs[-1]

    if struct not in _cache:
        _cache[struct] = _build(struct)
    nc = _cache[struct]

    # host-side shard prep (transposes, bf16 casts, weight slicing)
    KTb = [np.ascontiguousarray(K_seq[b].T).astype(BF16) for b in range(B)]
    VTb = [np.ascontiguousarray(V_seq[b].T).astype(BF16) for b in range(B)]
    QT = [np.ascontiguousarray(Q_seq[b].T).astype(BF16) for b in range(B)]
    vtb = np.stack([V_seq[b].T for b in range(B)]).astype(np.float32)
    wvm = (WV / float(L)).astype(np.float32)

    in_maps = []
    for c in range(N_CORES):
        qt = np.zeros((NU, 64, NQ), dtype=BF16)
        kt = np.zeros((S, 64, L), dtype=BF16)
        vt = np.zeros((S, 64, L), dtype=BF16)
        w = np.zeros((S, 64, 192), dtype=BF16)
        for s in range(S):
            job = assign[(c, s)]
            if job is None:
                continue
            b, h = job
            kt[s] = KTb[b]
            vt[s] = VTb[b]
            w[s, :, 0:64] = (WQ[:, h * d:(h + 1) * d] * scale).astype(BF16)
            w[s, :, 64:128] = WK[:, h * d:(h + 1) * d].astype(BF16)
            w[s, :, 128:192] = WV[:, h * d:(h + 1) * d].astype(BF16)
            for r, nqw in enumerate(struct[s]):
                q0 = min(r * NQ, L - nqw)
                qt[offs[s] + r, :, 0:nqw] = QT[b][:, q0:q0 + nqw]
        in_maps.append({"qt": qt, "kt": kt, "vt": vt, "w": w,
                        "vtb": vtb, "wvm": wvm})

    global _last_in_maps
    _last_in_maps = in_maps
    res = run_bass_kernel_spmd(nc, in_maps, core_ids=list(range(N_CORES)))
    results = res.results

    # gather
    out = np.zeros((B, L, H * d), dtype=np.float32)
    mo = results[0]["meanout"]  # [128, 8, B]
    mean_proj = np.transpose(mo, (2, 1, 0)).reshape(B, H * d)  # [B, H*d]
    for b in range(B):
        nq = min(max(q_len[b], 0), L)
        nh = min(max(v_len[b], 0), H)
        if nq > 0 and nh < H:
            out[b, :nq, nh * d:] = mean_proj[b, nh * d:][None, :]
    for (c, s), job in assign.items():
        if job is None:
            continue
        b, h = job
        nq = min(max(q_len[b], 0), L)
        for r, nqw in enumerate(struct[s]):
            q0 = min(r * NQ, L - nqw)
            lo, hi = q0, min(q0 + nqw, nq)
            if hi <= lo:
                continue
            out[b, lo:hi, h * d:(h + 1) * d] = \
                results[c]["out"][offs[s] + r, :hi - lo, :]
    return out


# revision 35
# speedup vs baseline: 1.1320x; 1.1320x over previous
"""Trainium2 Bass kernel for nn_Attention_7919919694519.

Multi-head attention (B=2, L=2048, H=16, d=64) with two data-dependent masks:
  - V_len[b] masks HEADS h >= V_len[b]: the reference adds -1e12 to every
    score of those heads, which collapses (in fp32) to a uniform softmax, so
    the masked head's output is mean_k(v) = (mean_k V_seq) @ WV_h  (rank-1).
  - Q_len[b] zeroes output rows q >= Q_len[b].

Strategy (host-visible Q_len/V_len drive the work list):
  - Only unmasked heads with live q rows do real attention. Each unmasked
    head is a "job" needing ceil(Q_len[b]/512) q-chunks (the last chunk
    trimmed to a 128-multiple of its live rows). Jobs are packed into
    head-slots dealt across 8 NeuronCores (SPMD: same NEFF, different
    data); K/V are projected once per slot, q-chunks stream through.
    No collectives; host scatters/gathers.
  - Per chunk on device: project q (bf16 matmul), scores S^T[k,q] in PSUM,
    exp on ScalarE (PSUM->SBUF bf16), AV accumulation with a ones-column
    appended to v so softmax denominators fall out of the same matmuls,
    PE transpose, reciprocal+scale on VectorE, DMA out. Emission is
    software-pipelined across chunk-units so ScalarE (the bottleneck
    engine) never starves at unit boundaries.
  - Masked-head rank-1 content: device reduces V_seq over k (VectorE) and
    projects through WV/2048; host broadcasts rows (pure output assembly).
"""

import math
import numpy as np
import ml_dtypes

import concourse.bass as bass
import concourse.tile as tile
from concourse import bacc, mybir
from concourse.bass_utils import run_bass_kernel_spmd
from concourse.masks import make_identity
from contextlib import ExitStack

BF16 = ml_dtypes.bfloat16
N_CORES = 8
B_, L_, D_, H_ = 2, 2048, 64, 16
NQ = 512              # max q rows per chunk
KT = 16               # number of 128-row k tiles (L/128)
SPS_FD = 1536         # score-psum slot free dim (3 banks)

_cache = {}


def _chunk_plan(nq):
    """k-tile sub-chunk sizes. Each k-tile gets its own 512-f32 PSUM lane so
    matmul outputs never cross a bank boundary (chunk = 3 banks)."""
    cl = 3
    out = [cl] * (KT // cl)
    if KT % cl:
        out = [KT % cl] + out
    return out


def _unit_order(struct):
    """Round-robin (slot, position) order; index = DRAM row in qt/out."""
    order = []
    max_r = max(len(w) for w in struct)
    for r in range(max_r):
        for s in range(len(struct)):
            if r < len(struct[s]):
                order.append((s, r))
    return order


def _build(struct):
    """Build + compile the SPMD NEFF.

    struct: tuple of per-slot tuples of chunk q-widths, e.g.
    ((512, 512, 512, 128), (512, 512, 256))."""
    nc = bacc.Bacc("TRN2", target_bir_lowering=False, debug=False,
                   num_devices=N_CORES)
    dt = mybir.dt
    S = len(struct)
    # interleave slots round-robin so slot prologues overlap earlier slots'
    # compute and the kernel tail lands on the smallest chunk. unit index u
    # equals its DRAM row in qt/out (host uses the same ordering).
    units = [(s, r == 0, struct[s][r]) for s, r in _unit_order(struct)]
    NU = len(units)

    qt_d = nc.dram_tensor("qt", [NU, 64, NQ], dt.bfloat16, kind="ExternalInput").ap()
    kt_d = nc.dram_tensor("kt", [S, 64, L_], dt.bfloat16, kind="ExternalInput").ap()
    vt_d = nc.dram_tensor("vt", [S, 64, L_], dt.bfloat16, kind="ExternalInput").ap()
    w_d = nc.dram_tensor("w", [S, 64, 128], dt.bfloat16, kind="ExternalInput").ap()
    vtb_d = nc.dram_tensor("vtb", [B_, 64, L_], dt.float32, kind="ExternalInput").ap()
    wvm_d = nc.dram_tensor("wvm", [64, H_ * 64], dt.float32, kind="ExternalInput").ap()
    out_d = nc.dram_tensor("out", [NU, 64, NQ], dt.float32, kind="ExternalOutput").ap()
    mo_d = nc.dram_tensor("meanout", [128, 8, B_], dt.float32, kind="ExternalOutput").ap()

    with tile.TileContext(nc) as tc, ExitStack() as ctx:
        sbufs = max(2, S)   # all slots' K/V live concurrently (interleaved)
        inp = ctx.enter_context(tc.tile_pool(name="inp", bufs=sbufs))
        proj = ctx.enter_context(tc.tile_pool(name="proj", bufs=sbufs))
        expp = ctx.enter_context(tc.tile_pool(name="expp", bufs=3))
        ob = ctx.enter_context(tc.tile_pool(name="ob", bufs=3))
        single = ctx.enter_context(tc.tile_pool(name="single", bufs=1))
        ps_s = ctx.enter_context(tc.tile_pool(name="ps_s", bufs=2, space="PSUM"))
        ps_a = ctx.enter_context(tc.tile_pool(name="ps_a", bufs=2, space="PSUM"))

        ones1 = single.tile([1, 64], dt.bfloat16)
        nc.vector.memset(ones1[:], 1.0)

        st = [dict() for _ in range(NU)]
        slot_tiles = {}

        def slot_k_prologue(u, on_act=False):
            # w DMA + tile allocation only: the K-projection is folded into
            # the q side via wqk = WQ_h @ WK_h^T / sqrt(d), so scores read
            # raw K^T. kt/vt DMAs are issued by slot_kv_dma (after the first
            # unit's qt DMA so the critical path leads the DMA queue).
            s, first, _ = units[u]
            if not first or s in slot_tiles:
                return
            w_sb = inp.tile([64, 128], dt.bfloat16, tag="w", name=f"w{s}")
            nc.sync.dma_start(w_sb[:], w_d[s])
            kt_sb = inp.tile([64, L_], dt.bfloat16, tag="kt", name=f"kt{s}")
            vt_sb = inp.tile([64, L_], dt.bfloat16, tag="vt", name=f"vt{s}")
            slot_tiles[s] = [w_sb, kt_sb, None, vt_sb]

        kv_dmad = set()

        def slot_kv_dma(u):
            s, first, _ = units[u]
            if not first or s in kv_dmad:
                return
            _, kt_sb, _, vt_sb = slot_tiles[s]
            nc.sync.dma_start(kt_sb[:], kt_d[s])
            nc.sync.dma_start(vt_sb[:], vt_d[s])
            kv_dmad.add(s)

        def slot_v_prologue(u):
            s, first, _ = units[u]
            if not first or slot_tiles[s][2] is not None:
                return
            w_sb, _, _, vt_sb = slot_tiles[s]
            # v projection into [k=128, 16, 65] layout (col 64 = ones)
            v_sb = proj.tile([128, KT, 65], dt.bfloat16, tag="v_sb")
            for half in range(2):
                vps = ps_s.tile([128, 8 * 64], dt.float32, tag="ps")
                for j in range(8):
                    t = half * 8 + j
                    nc.tensor.matmul(vps[:, j * 64:(j + 1) * 64],
                                     vt_sb[:, t * 128:(t + 1) * 128],
                                     w_sb[:, 64:128], start=True, stop=True)
                nc.vector.tensor_copy(
                    v_sb[:, half * 8:(half + 1) * 8, 0:64],
                    vps[:].rearrange("p (t d) -> p t d", t=8))
            nc.vector.memset(v_sb[:, :, 64], 1.0)
            slot_tiles[s][2] = v_sb

        def unit_prologue(u):
            s, _, nq = units[u]
            w_sb, kt_sb, v_sb, _ = slot_tiles[s]
            d = st[u]
            d["kTh"], d["s"] = kt_sb, s
            d["chunks"] = _chunk_plan(nq)
            d["offs"] = [sum(d["chunks"][:i]) for i in range(len(d["chunks"]) + 1)]
            d["nq"] = nq
            qt_sb = inp.tile([64, nq], dt.bfloat16, tag="qt", name=f"qt{u}")
            nc.sync.dma_start(qt_sb[:], qt_d[u][:, 0:nq])
            qps = ps_a.tile([64, nq], dt.float32, tag="pa", name=f"qps{u}")
            nc.tensor.matmul(qps[:], w_sb[:, 0:64], qt_sb[:],
                             start=True, stop=True)
            qTh = proj.tile([64, nq], dt.bfloat16, tag="qTh", name=f"qTh{u}")
            nc.vector.tensor_copy(qTh[:], qps[:])
            d["qTh"] = qTh
            d["sps"] = [None] * len(d["chunks"])
            d["ex"] = [None] * len(d["chunks"])

        def s_chunk(u, c):
            d = st[u]
            cl, nq = d["chunks"][c], d["nq"]
            sps = ps_s.tile([128, cl, 512], dt.float32, tag="ps", name=f"sps{u}_{c}")
            for j in range(cl):
                t = d["offs"][c] + j
                nc.tensor.matmul(sps[:, j, 0:nq],
                                 d["kTh"][:, t * 128:(t + 1) * 128],
                                 d["qTh"][:], start=True, stop=True)
            d["sps"][c] = sps

        def e_chunk(u, c):
            d = st[u]
            cl, nq = d["chunks"][c], d["nq"]
            ex = expp.tile([128, cl, nq], dt.bfloat16, tag="ex", name=f"ex{u}_{c}")
            nc.scalar.activation(ex[:], d["sps"][c][:, :, 0:nq],
                                 mybir.ActivationFunctionType.Exp)
            d["ex"][c] = ex

        def av_chunk(u, c):
            d = st[u]
            nq = d["nq"]
            v_sb = slot_tiles[d["s"]][2]
            if c == 0:
                d["av"] = ps_a.tile([65, nq], dt.float32, tag="pa", name=f"av{u}")
            for j in range(d["chunks"][c]):
                t = d["offs"][c] + j
                nc.tensor.matmul(d["av"][:], v_sb[:, t, :],
                                 d["ex"][c][:, j, :],
                                 start=(t == 0), stop=(t == KT - 1))

        def epilogue(u):
            # normalize in O^T layout: recip of the sums row, broadcast down
            # 64 partitions via a K=1 matmul, one multiply, one DMA. The
            # final [d, q] -> [q, d] transpose happens on the host (gather).
            d = st[u]
            nq = d["nq"]
            o_sb = ob.tile([65, nq], dt.float32, tag="o_sb", name=f"osb{u}")
            nc.vector.tensor_copy(o_sb[:], d["av"][:])
            rs = ob.tile([1, nq], dt.bfloat16, tag="rs", name=f"rs{u}")
            with nc.allow_low_precision(reason="softmax denominators are O(1e3); bf16 recip is plenty for the broadcast path"):
                nc.vector.reciprocal(rs[:], o_sb[64:65, :])
            rb = ps_a.tile([64, nq], dt.float32, tag="pa", name=f"rb{u}")
            nc.tensor.matmul(rb[:], ones1[:, 0:64], rs[:], start=True, stop=True)
            ot = ob.tile([64, nq], dt.float32, tag="ot", name=f"ot{u}")
            nc.vector.tensor_mul(ot[:], o_sb[0:64, :], rb[:])
            nc.sync.dma_start(out_d[u][:, 0:nq], ot[:])
            st[u] = None

        def mean_block():
            # masked-head rank-1 content: (sum_k V_seq) @ (WV/2048)
            wvm_sb = single.tile([64, H_ * 64], dt.float32)
            nc.sync.dma_start(wvm_sb[:], wvm_d[:])
            mvt = single.tile([64, B_], dt.float32)
            for b in range(B_):
                vtb_sb = inp.tile([64, L_], dt.float32, tag="vtb")
                nc.sync.dma_start(vtb_sb[:], vtb_d[b])
                nc.vector.reduce_sum(mvt[:, b:b + 1], vtb_sb[:],
                                     axis=mybir.AxisListType.X)
            mo_sb = single.tile([128, 8, B_], dt.float32)
            for c in range(8):
                mps = ps_a.tile([128, B_], dt.float32, tag="pa", name=f"mps{c}")
                nc.tensor.matmul(mps[:], wvm_sb[:, c * 128:(c + 1) * 128], mvt[:],
                                 start=True, stop=True)
                nc.vector.tensor_copy(mo_sb[:, c, :], mps[:])
            nc.sync.dma_start(mo_d[:], mo_sb[:])

        # software pipeline across chunk-units: the next unit's prologue and
        # first score chunk are emitted before this unit's AV tail/epilogue so
        # ScalarE never starves at unit boundaries.
        slot_k_prologue(0)
        unit_prologue(0)
        slot_kv_dma(0)
        s_chunk(0, 0)
        e_chunk(0, 0)
        # prefetch every other slot's K/V DMAs while unit 0 computes
        first_unit = {}
        for i, (s, first, _) in enumerate(units):
            if first:
                first_unit[s] = i
        for s in range(1, S):
            slot_k_prologue(first_unit[s])
            slot_kv_dma(first_unit[s])
        for u in range(NU):
            nch = len(st[u]["chunks"])
            for c in range(nch):
                if c + 1 < nch:
                    s_chunk(u, c + 1)
                    e_chunk(u, c + 1)
                    if c == max(0, nch - 2) and u + 1 < NU:
                        slot_k_prologue(u + 1)
                        slot_kv_dma(u + 1)
                        unit_prologue(u + 1)
                        s_chunk(u + 1, 0)
                        e_chunk(u + 1, 0)
                if c == 0:
                    slot_v_prologue(u)
                av_chunk(u, c)
            epilogue(u)
            if u == max(0, NU // 2 - 1):
                mean_block()

    nc.compile()
    return nc


def _round128(x):
    return max(128, (x + 127) // 128 * 128)


def _plan(q_len, v_len, B, L, H):
    """Pack unmasked-head jobs into head-slots.

    Returns (struct, assign): struct[s] = tuple of chunk q-widths;
    assign[(core, s)] = (b, h) or None."""
    jobs = []
    for b in range(B):
        nq = min(max(q_len[b], 0), L)
        nh = min(max(v_len[b], 0), H)
        if nq <= 0:
            continue
        r = (nq + NQ - 1) // NQ
        for h in range(nh):
            jobs.append((r, nq, b, h))
    jobs.sort(key=lambda x: (-x[0], -x[1]))
    n_slots = max(1, (len(jobs) + N_CORES - 1) // N_CORES)
    struct = []
    assign = {}
    for s in range(n_slots):
        col = jobs[s * N_CORES:(s + 1) * N_CORES]
        rmax = col[0][0] if col else 1
        widths = []
        for r in range(rmax):
            live = max((min(NQ, nq - r * NQ) for (jr, nq, _, _) in col
                        if r < jr), default=64)
            widths.append(int(live))
        struct.append(tuple(widths))
        for c in range(N_CORES):
            assign[(c, s)] = (col[c][2], col[c][3]) if c < len(col) else None
    return tuple(struct), assign


def kernel(Q_seq, K_seq, V_seq, WQ, WK, WV, Q_len, V_len):
    Q_seq = np.asarray(Q_seq, dtype=np.float32)
    K_seq = np.asarray(K_seq, dtype=np.float32)
    V_seq = np.asarray(V_seq, dtype=np.float32)
    WQ = np.asarray(WQ, dtype=np.float32)
    WK = np.asarray(WK, dtype=np.float32)
    WV = np.asarray(WV, dtype=np.float32)
    q_len = [int(x) for x in np.asarray(Q_len).reshape(-1)]
    v_len = [int(x) for x in np.asarray(V_len).reshape(-1)]
    B, L, d = Q_seq.shape
    H = WQ.shape[1] // d
    scale = 1.0 / math.sqrt(d)

    struct, assign = _plan(q_len, v_len, B, L, H)
    S = len(struct)
    order = _unit_order(struct)
    row_of = {sr: i for i, sr in enumerate(order)}
    NU = len(order)

    if struct not in _cache:
        _cache[struct] = _build(struct)
    nc = _cache[struct]

    # host-side shard prep (transposes, bf16 casts, weight slicing)
    KTb = [np.ascontiguousarray(K_seq[b].T).astype(BF16) for b in range(B)]
    VTb = [np.ascontiguousarray(V_seq[b].T).astype(BF16) for b in range(B)]
    QT = [np.ascontiguousarray(Q_seq[b].T).astype(BF16) for b in range(B)]
    vtb = np.stack([V_seq[b].T for b in range(B)]).astype(np.float32)
    wvm = (WV / float(L)).astype(np.float32)

    in_maps = []
    for c in range(N_CORES):
        qt = np.zeros((NU, 64, NQ), dtype=BF16)
        kt = np.zeros((S, 64, L), dtype=BF16)
        vt = np.zeros((S, 64, L), dtype=BF16)
        w = np.zeros((S, 64, 128), dtype=BF16)
        for s in range(S):
            job = assign[(c, s)]
            if job is None:
                continue
            b, h = job
            kt[s] = KTb[b]
            vt[s] = VTb[b]
            wq_h = WQ[:, h * d:(h + 1) * d]
            wk_h = WK[:, h * d:(h + 1) * d]
            w[s, :, 0:64] = (wq_h @ wk_h.T * scale).astype(BF16)
            w[s, :, 64:128] = WV[:, h * d:(h + 1) * d].astype(BF16)
            for r, nqw in enumerate(struct[s]):
                q0 = min(r * NQ, L - nqw)
                qt[row_of[(s, r)], :, 0:nqw] = QT[b][:, q0:q0 + nqw]
        in_maps.append({"qt": qt, "kt": kt, "vt": vt, "w": w,
                        "vtb": vtb, "wvm": wvm})

    global _last_in_maps
    _last_in_maps = in_maps
    res = run_bass_kernel_spmd(nc, in_maps, core_ids=list(range(N_CORES)))
    results = res.results

    # gather
    out = np.zeros((B, L, H * d), dtype=np.float32)
    mo = results[0]["meanout"]  # [128, 8, B]
    mean_proj = np.transpose(mo, (2, 1, 0)).reshape(B, H * d)  # [B, H*d]
    for b in range(B):
        nq = min(max(q_len[b], 0), L)
        nh = min(max(v_len[b], 0), H)
        if nq > 0 and nh < H:
            out[b, :nq, nh * d:] = mean_proj[b, nh * d:][None, :]
    for (c, s), job in assign.items():
        if job is None:
            continue
        b, h = job
        nq = min(max(q_len[b], 0), L)
        for r, nqw in enumerate(struct[s]):
            q0 = min(r * NQ, L - nqw)
            lo, hi = q0, min(q0 + nqw, nq)
            if hi <= lo:
                continue
            out[b, lo:hi, h * d:(h + 1) * d] = \
                results[c]["out"][row_of[(s, r)], :, :hi - lo].T
    return out
